# revision 37
# baseline (speedup 1.0000x reference)
"""Trainium2 Bass kernel for nn_Attention_83743272337693.

Quantized-attention transformer block:
  q/k/v projections -> RoPE(q,k) -> per-token-per-head int8 quantization of
  q,k -> int8 score GEMM -> causal softmax -> attn @ v -> o_proj.

Distribution (8 NeuronCores, SPMD): tensor-parallel over heads. Core c owns
query heads 4c..4c+3 and kv head c (GQA group). Wq/Wk/Wv are sharded
column-wise, Wo row-wise; each core computes a full [S, D] partial of the
output (stored f16) and the host sums the 8 partials (the all-reduce).

Single fused, software-pipelined phase. Per loop iteration i the PE queue
sees: [attention(i) rounds with o_proj(i-1) chunks as fillers, q-proj(i+1),
kv-proj(i+1), remaining o_proj(i-1) chunks, transposes(i+1)] so the PE never
head-of-line blocks on the DVE rope/quantize chain and the o_proj GEMM is
fully overlapped (no separate tail phase).

Profile-driven decisions (from NTFF traces of the 570us baseline):
- LDWEIGHTS issues ~2ns after the preceding matmul and runs hidden in the
  shadow weight buffer -- stationary reuse is NOT worth restructuring for.
  The real per-matmul cost is cols*0.417ns + ~45ns issue overhead.
- Scores are exp-throttled: exp on ACT takes ~774ns per [128,512] block,
  longer than the 258ns score matmul. psS is double-buffered and V/Z
  matmuls are interleaved into the score rounds so PE and ACT pipeline.
- DVE reciprocal is ~7.5ns/elem: the softmax normalization 1/Z now runs on
  ACT as Exp(-Ln(Z)) (2 ops per q-tile instead of 61us of DVE reciprocals).
- RoPE uses the rotate-half form o = x*[c;c] + rot(x)*[-s;s] where rot() is
  a negative-stride AP read, with the per-head table broadcast via 0-stride
  APs: 6 DVE ops per tile instead of 12, no host-side head replication.
- Transposes (bf16, PSUM) bitcast-share the attn@v accumulator bank; their
  lifetimes are disjoint. PSUM: proj 2, scores 2x1, Z 1, transp+attn@v 1,
  o_proj 2x1 = 8 banks.
"""
import numpy as np
import ml_dtypes

import concourse.bass as bass
import concourse.mybir as mybir
from concourse import bacc, bass_utils
from concourse.dve_ops import RECIPROCAL_APPROX_FAST, RECIP_APPROX_FAST_CONSTS
from concourse.tile import TileContext
from concourse.masks import make_identity

# Problem shape (hardcoded per contract).
B, S, D = 1, 2048, 4096
NH, NKV, HD = 32, 8, 128
N_CORES = 8
HQ = NH // N_CORES          # query heads per core (4)
ST = S // 128               # seq tiles (16)
KC = D // 128               # contraction chunks for projections (32)
HALF = HD // 2
SCALE = float(HD) ** -0.5
MAGIC = float(np.float32(1.5 * 2 ** 23))
MASK_VAL = -1.0e10
EXP_BIAS = -3.0

F32 = mybir.dt.float32
BF16 = mybir.dt.bfloat16
F16 = mybir.dt.float16
FP8 = mybir.dt.float8e4
LO_SCALE = 64.0  # lo-plane scale: keeps fp8 residual planes out of subnormals


def _rot_ap(ap4):
    """[p, h, 2, HALF] AP -> same with the '2' dim read in swapped order.

    Implements rot(x) = [x2; x1] as a negative-stride access pattern so the
    rotate-half RoPE form needs no data movement.
    """
    r = ap4.copy()
    st, num = r.ap[-2]
    assert num == 2
    r.ap[-2] = [-st, 2]
    r.offset += st
    return r


def build():
    nc = bacc.Bacc("TRN2", target_bir_lowering=False)

    xP = nc.dram_tensor("xP", [ST * 128, KC * 128], BF16, kind="ExternalInput")
    # rotate-half rope tables: cc2 = [c; c], ss2 = [-s; s] along hd (128)
    cc2P = nc.dram_tensor("cc2P", [128, ST * HD], F32, kind="ExternalInput")
    ss2P = nc.dram_tensor("ss2P", [128, ST * HD], F32, kind="ExternalInput")
    wqP = nc.dram_tensor("wqP", [128, KC * HQ * HD], BF16, kind="ExternalInput")
    wkvP = nc.dram_tensor("wkvP", [128, KC * 2 * HD], BF16, kind="ExternalInput")
    woP = nc.dram_tensor("woP", [128, HQ * D], F16, kind="ExternalInput")
    y = nc.dram_tensor("y", [S, D], F16, kind="ExternalOutput")

    mult = mybir.AluOpType.mult
    add = mybir.AluOpType.add

    with TileContext(nc) as tc:
        with (
            tc.tile_pool(name="persist", bufs=1) as persist,
            tc.tile_pool(name="small", bufs=4) as small,
            tc.tile_pool(name="xstream", bufs=2) as xpool,
            tc.tile_pool(name="ropebuf", bufs=2) as rpool,
            tc.tile_pool(name="zbuf", bufs=2) as zbuf,
            tc.tile_pool(name="obuf", bufs=3) as obuf,
            tc.tile_pool(name="psA", bufs=1, space="PSUM") as psA,
            tc.tile_pool(name="psS", bufs=2, space="PSUM") as psS,
            tc.tile_pool(name="psZ", bufs=1, space="PSUM") as psZ,
            tc.tile_pool(name="psTV", bufs=1, space="PSUM") as psTV,
            tc.tile_pool(name="psO", bufs=2, space="PSUM") as psO,
        ):
            # ---- persistent SBUF state ----
            qTs = persist.tile([128, HQ, S], BF16, tag="qTs")      # 2 MiB
            kTs = persist.tile([128, S], BF16, tag="kTs")          # 512 KiB
            v_sb = persist.tile([128, ST, HD], F16, tag="v_sb")    # 512 KiB
            ident_bf = persist.tile([128, 128], BF16, tag="ident_bf")
            maskT4 = persist.tile([128, HQ * 128], F32, tag="maskT4")
            ones_f16 = persist.tile([128, 128], F16, tag="ones_f16")
            ebias = persist.tile([128, 1], F32, tag="ebias")
            cc2 = persist.tile([128, ST, HD], F32, tag="cc2")      # 1 MiB
            ss2 = persist.tile([128, ST, HD], F32, tag="ss2")      # 1 MiB
            oh_all = persist.tile([128, ST, HQ * 128], F16, tag="oh_all")
            pT = persist.tile([128, ST, HQ, 128], F16, tag="pT")   # 2 MiB
            wq_sb = persist.tile([128, KC, HQ * HD], BF16, tag="wq_sb")   # 4 MiB
            wkv_sb = persist.tile([128, KC, 2 * HD], BF16, tag="wkv_sb")  # 2 MiB
            wo_sb = persist.tile([128, HQ, D], F16, tag="wo_sb")          # 4 MiB

            make_identity(nc, ident_bf[:])
            nc.gpsimd.memset(ones_f16[:], 1.0)
            nc.gpsimd.memset(ebias[:], EXP_BIAS)
            # Transposed causal mask, replicated for the 4 heads:
            # maskT[k, q] = 0 where q >= k else MASK_VAL (rows=k, cols=q).
            nc.gpsimd.memset(maskT4[:], 0.0)
            for h in range(HQ):
                nc.gpsimd.affine_select(
                    out=maskT4[:, h * 128:(h + 1) * 128],
                    in_=maskT4[:, h * 128:(h + 1) * 128],
                    compare_op=mybir.AluOpType.is_ge,
                    fill=MASK_VAL,
                    base=0,
                    pattern=[[1, 128]],
                    channel_multiplier=-1,
                )

            # ---- input DMAs, spread over the sync/scalar/gpsimd queues and
            # chunked/prioritized so the first-tile consumers start early ----
            wq_r = wqP.ap().rearrange("p (k n) -> p k n", k=KC)
            wkv_r = wkvP.ap().rearrange("p (k n) -> p k n", k=KC)
            cc2_r = cc2P.ap().rearrange("p (t d) -> p t d", t=ST)
            ss2_r = ss2P.ap().rearrange("p (t d) -> p t d", t=ST)
            wo_r = woP.ap().rearrange("p (h n) -> p h n", h=HQ)
            # sync carries only wq (first consumer); scalar carries wkv +
            # rope tables + half of wo; gpsimd gets x tiles + the other half
            # of wo (emitted in the prologue between x-tile DMAs)
            for lo, hi in [(0, 1), (1, 2), (2, 4), (4, 8), (8, 16), (16, 32)]:
                nc.sync.dma_start(wq_sb[:, lo:hi, :], wq_r[:, lo:hi, :])
            for lo, hi in [(0, 1), (1, 2)]:
                nc.scalar.dma_start(wkv_sb[:, lo:hi, :], wkv_r[:, lo:hi, :])
            nc.scalar.dma_start(cc2[:, :4, :], cc2_r[:, :4, :])
            nc.scalar.dma_start(ss2[:, :4, :], ss2_r[:, :4, :])
            for lo, hi in [(2, 4), (4, 8), (8, 16), (16, 32)]:
                nc.scalar.dma_start(wkv_sb[:, lo:hi, :], wkv_r[:, lo:hi, :])
            nc.scalar.dma_start(cc2[:, 4:, :], cc2_r[:, 4:, :])
            nc.scalar.dma_start(ss2[:, 4:, :], ss2_r[:, 4:, :])
            nc.scalar.dma_start(wo_sb[:, :, :2048], wo_r[:, :, :2048])

            xts = {}

            def emit_x_dma(st):
                if st >= ST:
                    return
                xt = xpool.tile([128, KC, 128], BF16, tag="xt")
                xts[st] = xt
                rows = slice(st * 128, (st + 1) * 128)
                src = xP.ap()[rows, :].rearrange("p (k s) -> p k s", k=KC)
                for q in range(4):
                    nc.gpsimd.dma_start(xt[:, q * 8:(q + 1) * 8, :],
                                        src[:, q * 8:(q + 1) * 8, :])

            psqs = {}

            def emit_proj_q(st):
                ps_q = psA.tile([128, HQ * HD], F32, tag="ps_q")
                psqs[st] = ps_q
                xt = xts[st]
                for kc in range(KC):
                    nc.tensor.matmul(ps_q[:], xt[:, kc, :], wq_sb[:, kc, :],
                                     start=(kc == 0), stop=(kc == KC - 1))

            pskvs = {}

            def emit_proj_kv(st):
                ps_kv = psA.tile([128, 2 * HD], F32, tag="ps_kv")
                pskvs[st] = ps_kv
                xt = xts[st]
                for kc in range(KC):
                    nc.tensor.matmul(ps_kv[:], xt[:, kc, :], wkv_sb[:, kc, :],
                                     start=(kc == 0), stop=(kc == KC - 1))

            ropes = {}

            def emit_rope_q(st):
                # rotate-half: o = x*cc2 + rot(x)*ss2, batched over 4 heads
                # with 0-stride head-broadcast tables.
                rope = rpool.tile([128, HQ + 1, HD], F32, tag="rope")
                tmpq = rpool.tile([128, HQ, HD], F32, tag="tmpq")
                ropes[st] = rope
                q_src = psqs.pop(st)
                q4 = q_src[:].rearrange("p (h two f) -> p h two f", h=HQ, two=2)
                cc_b = cc2[:, st, :].unsqueeze(1).broadcast_to([128, HQ, HD])
                ss_b = ss2[:, st, :].unsqueeze(1).broadcast_to([128, HQ, HD])
                ro_q = rope[:, :HQ, :]
                nc.vector.tensor_tensor(
                    tmpq[:].rearrange("p h (two f) -> p h two f", two=2),
                    _rot_ap(q4),
                    ss_b.rearrange("p h (two f) -> p h two f", two=2),
                    op=mult)
                nc.vector.tensor_tensor(
                    ro_q, q4.rearrange("p h two f -> p h (two f)"),
                    cc_b, op=mult)
                nc.vector.tensor_tensor(ro_q, ro_q, tmpq[:], op=add)

            def emit_rope_k_v(st):
                rope = ropes[st]
                ps_kv = pskvs.pop(st)
                # v: straight cast to fp16 (no rope/quant) on ACT
                nc.scalar.copy(v_sb[:, st, :], ps_kv[:, HD:2 * HD])
                k2 = ps_kv[:, :HD].rearrange("p (two f) -> p two f", two=2)
                tmpk = rpool.tile([128, HD], F32, tag="tmpk")
                ro_k = rope[:, HQ, :]
                nc.vector.tensor_tensor(
                    tmpk[:], _rot_ap(k2),
                    ss2[:, st, :].rearrange("p (two f) -> p two f", two=2),
                    op=mult)
                nc.vector.tensor_tensor(
                    ro_k, ps_kv[:, :HD], cc2[:, st, :], op=mult)
                nc.vector.tensor_tensor(ro_k, ro_k, tmpk[:], op=add)

            qks = {}

            def emit_quant(st):
                # q~ = round(q*127/am) * (am*SCALE/127), k~ likewise without
                # SCALE; round() via the magic-constant trick.
                rope = ropes.pop(st)
                am = small.tile([128, HQ + 1], F32, tag="am")
                nc.vector.tensor_reduce(am[:], rope[:],
                                        axis=mybir.AxisListType.X,
                                        op=mybir.AluOpType.max,
                                        apply_absolute_value=True)
                nc.vector.tensor_scalar_max(am[:], am[:], 1e-5)
                sc = small.tile([128, HQ + 1], F32, tag="sc")
                nc.vector.reciprocal(sc[:], am[:])
                nc.vector.tensor_scalar_mul(sc[:], sc[:], 127.0)
                rs = small.tile([128, HQ + 1], F32, tag="rs")
                nc.vector.tensor_scalar(rs[:, :HQ], am[:, :HQ], SCALE / 127.0,
                                        None, op0=mult)
                nc.vector.tensor_scalar(rs[:, HQ:], am[:, HQ:], 1.0 / 127.0,
                                        None, op0=mult)
                qk = rpool.tile([128, HQ + 1, HD], BF16, tag="qk")
                qks[st] = qk
                rnd = rpool.tile([128, HD], F32, tag="rnd")
                for hh in range(HQ + 1):
                    nc.vector.tensor_scalar(rnd[:], rope[:, hh, :],
                                            sc[:, hh:hh + 1], MAGIC,
                                            op0=mult, op1=add)
                    nc.vector.tensor_scalar(qk[:, hh, :], rnd[:], -MAGIC,
                                            rs[:, hh:hh + 1],
                                            op0=add, op1=mult)

            shs = {}

            def emit_transp(st):
                # bf16 PE transposes into [hd, seq]; the 5 heads (1280B) live
                # in a bitcast view of the attn@v f32 accumulator bank.
                sh = psTV.tile([128, 512], F32, tag="sh")
                shs[st] = sh
                qk = qks.pop(st)
                shb = sh[:].bitcast(BF16).rearrange(
                    "p (h f) -> p h f", h=8)
                for hh in range(HQ + 1):
                    nc.tensor.transpose(shb[:, hh, :], qk[:, hh, :],
                                        ident_bf[:])
                rows = slice(st * 128, (st + 1) * 128)
                nc.scalar.copy(
                    qTs[:, :, rows],
                    shb[:, :HQ, :])
                nc.scalar.copy(kTs[:, rows], shb[:, HQ, :])

            oh_r = oh_all[:].rearrange("p t (h q) -> p t h q", h=HQ)
            out_ts = {}

            def oproj_chunk(qt, j):
                def emit():
                    # chunks 0-3 fill out_t half 0, 4-7 half 1; one big
                    # 4KB-per-row y DMA per half.
                    if j % 4 == 0:
                        out_ts[qt] = obuf.tile([128, 2048], F16,
                                               name="out_t", tag="out_t")
                    out_t = out_ts[qt]
                    ps_O = psO.tile([128, 512], F32, tag="ps_O")
                    for h in range(HQ):
                        nc.tensor.matmul(
                            ps_O[:], oh_r[:, qt, h, :],
                            wo_sb[:, h, j * 512:(j + 1) * 512],
                            start=(h == 0), stop=(h == HQ - 1))
                    dst = out_t[:, (j % 4) * 512:(j % 4 + 1) * 512]
                    # ACT drains the last tiles' exps at the very end; route
                    # the final o_proj copies to the then-idle DVE instead
                    if qt >= ST - 3 or j % 2 == 0:
                        nc.vector.tensor_copy(dst, ps_O[:])
                    else:
                        nc.scalar.copy(dst, ps_O[:])
                    if j % 2 == 1:
                        qtr = j // 2
                        deng = nc.sync if qtr % 2 == 0 else nc.gpsimd
                        deng.dma_start(
                            y.ap()[qt * 128:(qt + 1) * 128,
                                   qtr * 1024:(qtr + 1) * 1024],
                            out_t[:, (j % 4 - 1) * 512:(j % 4 + 1) * 512])
                return emit

            def emit_attention(qt, fillers):
                qcols = slice(qt * 128, (qt + 1) * 128)
                nblk = qt + 1
                sh = shs.pop(qt)          # f32 view: attn@v accumulator
                ps_z = psZ.tile([128, HQ * 128], F32, tag="ps_z")

                def emit_S(kc):
                    ps_S = psS.tile([128, HQ * 128], F32, tag="ps_S")
                    nc.tensor.matmul(ps_S[:],
                                     kTs[:, kc * 128:(kc + 1) * 128],
                                     qTs[:, :, qcols])
                    if kc == qt:
                        nc.vector.tensor_tensor(ps_S[:], ps_S[:], maskT4[:],
                                                op=add)
                    nc.scalar.activation(
                        pT[:, kc, :, :],
                        ps_S[:].rearrange("p (h q) -> p h q", h=HQ),
                        mybir.ActivationFunctionType.Exp, bias=ebias[:])

                def emit_VZ(kc, first, last):
                    rhs = pT[:, kc, :, :].rearrange("p h q -> p (h q)")
                    nc.tensor.matmul(sh[:], v_sb[:, kc, :], rhs,
                                     start=first, stop=last)
                    nc.tensor.matmul(ps_z[:], ones_f16[:], rhs,
                                     start=first, stop=last)

                # diagonal block first: its mask-add + exp latency hides
                # behind the subsequent rounds. V/Z trail the scores by two
                # rounds so they never wait on the exp.
                ks = [qt] + list(range(nblk - 1))
                vz = 0
                emit_S(ks[0])
                for r in range(1, nblk):
                    emit_S(ks[r])
                    if r >= 2:
                        emit_VZ(ks[vz], first=(vz == 0), last=False)
                        vz += 1
                    if fillers and r % 2 == 1:
                        fillers.pop(0)()
                while vz < nblk:
                    emit_VZ(ks[vz], first=(vz == 0), last=(vz == nblk - 1))
                    vz += 1

                # normalization: zinv via 1-instruction approx reciprocal
                # (~51 ULP) on DVE, then one DVE mult
                zinv = zbuf.tile([128, HQ * 128], F32, tag="zinv")
                c = RECIP_APPROX_FAST_CONSTS
                nc.vector._custom_dve(
                    RECIPROCAL_APPROX_FAST, out=zinv[:], in0=ps_z[:],
                    s0=c["s0"], s1=c["s1"], imm2=c["imm2"])
                nc.vector.tensor_tensor(oh_all[:, qt, :], sh[:], zinv[:],
                                        op=mult)

            # ---- prologue: tile 0 through transpose ----
            emit_x_dma(0)
            emit_x_dma(1)
            nc.gpsimd.dma_start(wo_sb[:, :, 2048:], wo_r[:, :, 2048:])
            emit_proj_q(0)
            emit_rope_q(0)
            emit_proj_kv(0)
            emit_rope_k_v(0)
            emit_quant(0)
            emit_transp(0)

            # ---- fused pipeline ----
            for i in range(ST):
                fillers = [oproj_chunk(i - 1, j) for j in range(8)] \
                    if i >= 1 else []
                last = i == ST - 1
                emit_attention(i, [] if last else fillers)
                emit_x_dma(i + 2)
                if not last:
                    emit_proj_q(i + 1)
                    emit_rope_q(i + 1)
                    emit_proj_kv(i + 1)
                    emit_rope_k_v(i + 1)
                    emit_quant(i + 1)
                for f in fillers[:4]:
                    f()
                del fillers[:4]
                if not last:
                    emit_transp(i + 1)
                for f in fillers:
                    f()
                fillers.clear()
            # epilogue: last o_proj
            for j in range(8):
                oproj_chunk(ST - 1, j)()

    nc.finalize()
    return nc


_NC_CACHE = None


def _get_nc():
    global _NC_CACHE
    if _NC_CACHE is None:
        _NC_CACHE = build()
    return _NC_CACHE


def make_in_maps(x, cos, sin, Wq, Wk, Wv, Wo):
    """Shard + pre-transpose the full inputs into the 8 per-core maps.

    All layouts give the DMA large contiguous per-partition segments:
    xP[st*128+p, kc*128+s] = x[st*128+s, kc*128+p]; weights are [128, ...]
    with the SBUF-destination layout materialized host-side. The rope
    tables are packed rotate-half style: cc2 = [c; c], ss2 = [-s; s].
    """
    bf16 = ml_dtypes.bfloat16
    x = np.asarray(x, np.float32)
    xP = np.ascontiguousarray(
        x.reshape(ST, 128, KC, 128).transpose(0, 3, 2, 1)
        .reshape(ST * 128, KC * 128)).astype(bf16)
    cos = np.asarray(cos, np.float32)
    sin = np.asarray(sin, np.float32)

    def prep(t):  # [S, HD] -> [128, ST*HD]
        r = t.reshape(ST, 128, HD).transpose(1, 0, 2)
        return np.ascontiguousarray(r.reshape(128, ST * HD))

    cc2 = prep(np.concatenate([cos, cos], axis=1))
    ss2 = prep(np.concatenate([-sin, sin], axis=1))
    Wq = np.asarray(Wq, np.float32)
    Wk = np.asarray(Wk, np.float32)
    Wv = np.asarray(Wv, np.float32)
    Wo = np.asarray(Wo, np.float32)
    in_maps = []
    for c in range(N_CORES):
        qs = slice(c * HQ * HD, (c + 1) * HQ * HD)
        ks = slice(c * HD, (c + 1) * HD)
        wq_c = Wq[:, qs].reshape(KC, 128, HQ * HD).transpose(1, 0, 2)
        wkv_c = np.concatenate([Wk[:, ks], Wv[:, ks]], axis=1) \
            .reshape(KC, 128, 2 * HD).transpose(1, 0, 2)
        wo_c = Wo[qs, :].reshape(HQ, 128, D).transpose(1, 0, 2)
        in_maps.append({
            "xP": xP,
            "cc2P": cc2,
            "ss2P": ss2,
            "wqP": np.ascontiguousarray(
                wq_c.reshape(128, KC * HQ * HD)).astype(bf16),
            "wkvP": np.ascontiguousarray(
                wkv_c.reshape(128, KC * 2 * HD)).astype(bf16),
            "woP": np.ascontiguousarray(
                wo_c.reshape(128, HQ * D)).astype(np.float16),
        })
    return in_maps


def run(x, cos, sin, Wq, Wk, Wv, Wo, trace=False):
    nc = _get_nc()
    in_maps = make_in_maps(x, cos, sin, Wq, Wk, Wv, Wo)
    res = bass_utils.run_bass_kernel_spmd(
        nc, in_maps, core_ids=list(range(N_CORES)), trace=trace)
    partials = np.stack([res.results[c]["y"].astype(np.float32)
                         for c in range(N_CORES)])
    out = partials.sum(axis=0)
    return out.reshape(B, S, D), res


def kernel(x, cos, sin, Wq, Wk, Wv, Wo):
    out, _ = run(x, cos, sin, Wq, Wk, Wv, Wo, trace=False)
    return out


# revision 39
# speedup vs baseline: 1.0193x; 1.0193x over previous
"""Trainium2 Bass kernel for nn_Attention_83743272337693.

Quantized-attention transformer block:
  q/k/v projections -> RoPE(q,k) -> per-token-per-head int8 quantization of
  q,k -> int8 score GEMM -> causal softmax -> attn @ v -> o_proj.

Distribution (8 NeuronCores, SPMD): tensor-parallel over heads. Core c owns
query heads 4c..4c+3 and kv head c (GQA group). Wq/Wk/Wv are sharded
column-wise, Wo row-wise; each core computes a full [S, D] partial of the
output (stored f16) and the host sums the 8 partials (the all-reduce).

Single fused, software-pipelined phase. Per loop iteration i the PE queue
sees: [attention(i) rounds with o_proj(i-1) chunks as fillers, q-proj(i+1),
kv-proj(i+1), remaining o_proj(i-1) chunks, transposes(i+1)] so the PE never
head-of-line blocks on the DVE rope/quantize chain and the o_proj GEMM is
fully overlapped (no separate tail phase).

Profile-driven decisions (from NTFF traces of the 570us baseline):
- LDWEIGHTS issues ~2ns after the preceding matmul and runs hidden in the
  shadow weight buffer -- stationary reuse is NOT worth restructuring for.
  The real per-matmul cost is cols*0.417ns + ~45ns issue overhead.
- Scores are exp-throttled: exp on ACT takes ~774ns per [128,512] block,
  longer than the 258ns score matmul. psS is double-buffered and V/Z
  matmuls are interleaved into the score rounds so PE and ACT pipeline.
- DVE reciprocal is ~7.5ns/elem and ACT Ln/Exp thrashes the activation
  table (1.3us reload per function switch): the softmax 1/Z runs as the
  1-instruction RECIPROCAL_APPROX_FAST custom DVE op (~51 ULP).
- fp8e4 DoubleRow was tried for the projections (3-term hi/lo split) and
  REVERTED: on hw a DoubleRow matmul issues at the same 213ns/512-col as
  bf16 (2x MACs per instruction, not the cost model's 0.5 cyc/row), so the
  3-term scheme's 48 instructions lose to bf16's 32 despite better
  accuracy (1.01e-2 vs 1.33e-2 rel_l2).
- RoPE uses the rotate-half form o = x*[c;c] + rot(x)*[-s;s] where rot() is
  a negative-stride AP read, with the per-head table broadcast via 0-stride
  APs: 6 DVE ops per tile instead of 12, no host-side head replication.
- Transposes (bf16, PSUM) bitcast-share the attn@v accumulator bank; their
  lifetimes are disjoint. PSUM: proj 2, scores 2x1, Z 1, transp+attn@v 1,
  o_proj 2x1 = 8 banks.
"""
import numpy as np
import ml_dtypes

import concourse.bass as bass
import concourse.mybir as mybir
from concourse import bacc, bass_utils
from concourse.dve_ops import RECIPROCAL_APPROX_FAST, RECIP_APPROX_FAST_CONSTS
from concourse.tile import TileContext
from concourse.masks import make_identity

# Problem shape (hardcoded per contract).
B, S, D = 1, 2048, 4096
NH, NKV, HD = 32, 8, 128
N_CORES = 8
HQ = NH // N_CORES          # query heads per core (4)
ST = S // 128               # seq tiles (16)
KC = D // 128               # contraction chunks for projections (32)
HALF = HD // 2
SCALE = float(HD) ** -0.5
MAGIC = float(np.float32(1.5 * 2 ** 23))
MASK_VAL = -1.0e10
EXP_BIAS = -3.0

F32 = mybir.dt.float32
BF16 = mybir.dt.bfloat16
F16 = mybir.dt.float16
FP8 = mybir.dt.float8e4
LO_SCALE = 64.0  # lo-plane scale: keeps fp8 residual planes out of subnormals


def _rot_ap(ap4):
    """[p, h, 2, HALF] AP -> same with the '2' dim read in swapped order.

    Implements rot(x) = [x2; x1] as a negative-stride access pattern so the
    rotate-half RoPE form needs no data movement.
    """
    r = ap4.copy()
    st, num = r.ap[-2]
    assert num == 2
    r.ap[-2] = [-st, 2]
    r.offset += st
    return r


def build():
    nc = bacc.Bacc("TRN2", target_bir_lowering=False)

    xP = nc.dram_tensor("xP", [ST * 128, KC * 128], BF16, kind="ExternalInput")
    # rotate-half rope tables: cc2 = [c; c], ss2 = [-s; s] along hd (128)
    cc2P = nc.dram_tensor("cc2P", [128, ST * HD], F32, kind="ExternalInput")
    ss2P = nc.dram_tensor("ss2P", [128, ST * HD], F32, kind="ExternalInput")
    wqP = nc.dram_tensor("wqP", [128, KC * HQ * HD], BF16, kind="ExternalInput")
    wkvP = nc.dram_tensor("wkvP", [128, KC * 2 * HD], BF16, kind="ExternalInput")
    woP = nc.dram_tensor("woP", [128, HQ * D], F16, kind="ExternalInput")
    y = nc.dram_tensor("y", [S, D], F16, kind="ExternalOutput")

    mult = mybir.AluOpType.mult
    add = mybir.AluOpType.add

    with TileContext(nc) as tc:
        with (
            tc.tile_pool(name="persist", bufs=1) as persist,
            tc.tile_pool(name="small", bufs=4) as small,
            tc.tile_pool(name="xstream", bufs=2) as xpool,
            tc.tile_pool(name="ropebuf", bufs=2) as rpool,
            tc.tile_pool(name="zbuf", bufs=2) as zbuf,
            tc.tile_pool(name="obuf", bufs=3) as obuf,
            tc.tile_pool(name="psA", bufs=1, space="PSUM") as psA,
            tc.tile_pool(name="psS", bufs=2, space="PSUM") as psS,
            tc.tile_pool(name="psZ", bufs=1, space="PSUM") as psZ,
            tc.tile_pool(name="psTV", bufs=1, space="PSUM") as psTV,
            tc.tile_pool(name="psO", bufs=2, space="PSUM") as psO,
        ):
            # ---- persistent SBUF state ----
            qTs = persist.tile([128, HQ, S], BF16, tag="qTs")      # 2 MiB
            kTs = persist.tile([128, S], BF16, tag="kTs")          # 512 KiB
            v_sb = persist.tile([128, ST, HD], F16, tag="v_sb")    # 512 KiB
            ident_bf = persist.tile([128, 128], BF16, tag="ident_bf")
            maskT4 = persist.tile([128, HQ * 128], F32, tag="maskT4")
            ones_f16 = persist.tile([128, 128], F16, tag="ones_f16")
            ebias = persist.tile([128, 1], F32, tag="ebias")
            cc2 = persist.tile([128, ST, HD], F32, tag="cc2")      # 1 MiB
            ss2 = persist.tile([128, ST, HD], F32, tag="ss2")      # 1 MiB
            oh_all = persist.tile([128, ST, HQ * 128], F16, tag="oh_all")
            pT = persist.tile([128, ST, HQ, 128], F16, tag="pT")   # 2 MiB
            wq_sb = persist.tile([128, KC, HQ * HD], BF16, tag="wq_sb")   # 4 MiB
            wkv_sb = persist.tile([128, KC, 2 * HD], BF16, tag="wkv_sb")  # 2 MiB
            wo_sb = persist.tile([128, HQ, D], F16, tag="wo_sb")          # 4 MiB

            make_identity(nc, ident_bf[:])
            nc.gpsimd.memset(ones_f16[:], 1.0)
            nc.gpsimd.memset(ebias[:], EXP_BIAS)
            # Transposed causal mask, replicated for the 4 heads:
            # maskT[k, q] = 0 where q >= k else MASK_VAL (rows=k, cols=q).
            nc.gpsimd.memset(maskT4[:], 0.0)
            for h in range(HQ):
                nc.gpsimd.affine_select(
                    out=maskT4[:, h * 128:(h + 1) * 128],
                    in_=maskT4[:, h * 128:(h + 1) * 128],
                    compare_op=mybir.AluOpType.is_ge,
                    fill=MASK_VAL,
                    base=0,
                    pattern=[[1, 128]],
                    channel_multiplier=-1,
                )

            # ---- input DMAs, spread over the sync/scalar/gpsimd queues and
            # chunked/prioritized so the first-tile consumers start early ----
            wq_r = wqP.ap().rearrange("p (k n) -> p k n", k=KC)
            wkv_r = wkvP.ap().rearrange("p (k n) -> p k n", k=KC)
            cc2_r = cc2P.ap().rearrange("p (t d) -> p t d", t=ST)
            ss2_r = ss2P.ap().rearrange("p (t d) -> p t d", t=ST)
            wo_r = woP.ap().rearrange("p (h n) -> p h n", h=HQ)
            # sync carries only wq (first consumer); scalar carries wkv +
            # rope tables + half of wo; gpsimd gets x tiles + the other half
            # of wo (emitted in the prologue between x-tile DMAs)
            for lo, hi in [(0, 1), (1, 2), (2, 4), (4, 8), (8, 16), (16, 32)]:
                nc.sync.dma_start(wq_sb[:, lo:hi, :], wq_r[:, lo:hi, :])
            for lo, hi in [(0, 1), (1, 2)]:
                nc.scalar.dma_start(wkv_sb[:, lo:hi, :], wkv_r[:, lo:hi, :])
            nc.scalar.dma_start(cc2[:, :4, :], cc2_r[:, :4, :])
            nc.scalar.dma_start(ss2[:, :4, :], ss2_r[:, :4, :])
            for lo, hi in [(2, 4), (4, 8), (8, 16), (16, 32)]:
                nc.scalar.dma_start(wkv_sb[:, lo:hi, :], wkv_r[:, lo:hi, :])
            nc.scalar.dma_start(cc2[:, 4:, :], cc2_r[:, 4:, :])
            nc.scalar.dma_start(ss2[:, 4:, :], ss2_r[:, 4:, :])
            nc.scalar.dma_start(wo_sb[:, :, :2048], wo_r[:, :, :2048])

            xts = {}

            def emit_x_dma(st):
                if st >= ST:
                    return
                xt = xpool.tile([128, KC, 128], BF16, tag="xt")
                xts[st] = xt
                rows = slice(st * 128, (st + 1) * 128)
                src = xP.ap()[rows, :].rearrange("p (k s) -> p k s", k=KC)
                for q in range(4):
                    nc.gpsimd.dma_start(xt[:, q * 8:(q + 1) * 8, :],
                                        src[:, q * 8:(q + 1) * 8, :])

            psqs = {}

            def emit_proj_q(st):
                ps_q = psA.tile([128, HQ * HD], F32, tag="ps_q")
                psqs[st] = ps_q
                xt = xts[st]
                for kc in range(KC):
                    nc.tensor.matmul(ps_q[:], xt[:, kc, :], wq_sb[:, kc, :],
                                     start=(kc == 0), stop=(kc == KC - 1))

            pskvs = {}

            def emit_proj_kv(st):
                ps_kv = psA.tile([128, 2 * HD], F32, tag="ps_kv")
                pskvs[st] = ps_kv
                xt = xts[st]
                for kc in range(KC):
                    nc.tensor.matmul(ps_kv[:], xt[:, kc, :], wkv_sb[:, kc, :],
                                     start=(kc == 0), stop=(kc == KC - 1))

            ropes = {}

            def emit_rope_q(st):
                # rotate-half: o = x*cc2 + rot(x)*ss2, batched over 4 heads
                # with 0-stride head-broadcast tables.
                rope = rpool.tile([128, HQ + 1, HD], F32, tag="rope")
                tmpq = rpool.tile([128, HQ, HD], F32, tag="tmpq")
                ropes[st] = rope
                q_src = psqs.pop(st)
                q4 = q_src[:].rearrange("p (h two f) -> p h two f", h=HQ, two=2)
                cc_b = cc2[:, st, :].unsqueeze(1).broadcast_to([128, HQ, HD])
                ss_b = ss2[:, st, :].unsqueeze(1).broadcast_to([128, HQ, HD])
                ro_q = rope[:, :HQ, :]
                nc.vector.tensor_tensor(
                    tmpq[:].rearrange("p h (two f) -> p h two f", two=2),
                    _rot_ap(q4),
                    ss_b.rearrange("p h (two f) -> p h two f", two=2),
                    op=mult)
                nc.vector.tensor_tensor(
                    ro_q, q4.rearrange("p h two f -> p h (two f)"),
                    cc_b, op=mult)
                nc.vector.tensor_tensor(ro_q, ro_q, tmpq[:], op=add)

            def emit_rope_k_v(st):
                rope = ropes[st]
                ps_kv = pskvs.pop(st)
                # v: straight cast to fp16 (no rope/quant) on ACT
                nc.scalar.copy(v_sb[:, st, :], ps_kv[:, HD:2 * HD])
                k2 = ps_kv[:, :HD].rearrange("p (two f) -> p two f", two=2)
                tmpk = rpool.tile([128, HD], F32, tag="tmpk")
                ro_k = rope[:, HQ, :]
                nc.vector.tensor_tensor(
                    tmpk[:], _rot_ap(k2),
                    ss2[:, st, :].rearrange("p (two f) -> p two f", two=2),
                    op=mult)
                nc.vector.tensor_tensor(
                    ro_k, ps_kv[:, :HD], cc2[:, st, :], op=mult)
                nc.vector.tensor_tensor(ro_k, ro_k, tmpk[:], op=add)

            qks = {}

            def emit_quant(st):
                # q~ = round(q*127/am) * (am*SCALE/127), k~ likewise without
                # SCALE; round() via the magic-constant trick.
                rope = ropes.pop(st)
                am = small.tile([128, HQ + 1], F32, tag="am")
                nc.vector.tensor_reduce(am[:], rope[:],
                                        axis=mybir.AxisListType.X,
                                        op=mybir.AluOpType.max,
                                        apply_absolute_value=True)
                nc.vector.tensor_scalar_max(am[:], am[:], 1e-5)
                sc = small.tile([128, HQ + 1], F32, tag="sc")
                nc.vector.reciprocal(sc[:], am[:])
                nc.vector.tensor_scalar_mul(sc[:], sc[:], 127.0)
                rs = small.tile([128, HQ + 1], F32, tag="rs")
                nc.vector.tensor_scalar(rs[:, :HQ], am[:, :HQ], SCALE / 127.0,
                                        None, op0=mult)
                nc.vector.tensor_scalar(rs[:, HQ:], am[:, HQ:], 1.0 / 127.0,
                                        None, op0=mult)
                qk = rpool.tile([128, HQ + 1, HD], BF16, tag="qk")
                qks[st] = qk
                rnd = rpool.tile([128, HD], F32, tag="rnd")
                for hh in range(HQ + 1):
                    nc.vector.tensor_scalar(rnd[:], rope[:, hh, :],
                                            sc[:, hh:hh + 1], MAGIC,
                                            op0=mult, op1=add)
                    nc.vector.tensor_scalar(qk[:, hh, :], rnd[:], -MAGIC,
                                            rs[:, hh:hh + 1],
                                            op0=add, op1=mult)

            shs = {}

            def emit_transp(st):
                # bf16 PE transposes into [hd, seq]; the 5 heads (1280B) live
                # in a bitcast view of the attn@v f32 accumulator bank.
                sh = psTV.tile([128, 512], F32, tag="sh")
                shs[st] = sh
                qk = qks.pop(st)
                shb = sh[:].bitcast(BF16).rearrange(
                    "p (h f) -> p h f", h=8)
                for hh in range(HQ + 1):
                    nc.tensor.transpose(shb[:, hh, :], qk[:, hh, :],
                                        ident_bf[:])
                rows = slice(st * 128, (st + 1) * 128)
                nc.scalar.copy(
                    qTs[:, :, rows],
                    shb[:, :HQ, :])
                nc.scalar.copy(kTs[:, rows], shb[:, HQ, :])

            oh_r = oh_all[:].rearrange("p t (h q) -> p t h q", h=HQ)
            out_ts = {}

            def oproj_chunk(qt, j):
                def emit():
                    # chunks 0-3 fill out_t half 0, 4-7 half 1; one big
                    # 4KB-per-row y DMA per half.
                    if j % 4 == 0:
                        out_ts[qt] = obuf.tile([128, 2048], F16,
                                               name="out_t", tag="out_t")
                    out_t = out_ts[qt]
                    ps_O = psO.tile([128, 512], F32, tag="ps_O")
                    for h in range(HQ):
                        nc.tensor.matmul(
                            ps_O[:], oh_r[:, qt, h, :],
                            wo_sb[:, h, j * 512:(j + 1) * 512],
                            start=(h == 0), stop=(h == HQ - 1))
                    dst = out_t[:, (j % 4) * 512:(j % 4 + 1) * 512]
                    if j % 2 == 0:
                        nc.vector.tensor_copy(dst, ps_O[:])
                    else:
                        nc.scalar.copy(dst, ps_O[:])
                    if j % 2 == 1:
                        qtr = j // 2
                        deng = nc.sync if qtr % 2 == 0 else nc.gpsimd
                        deng.dma_start(
                            y.ap()[qt * 128:(qt + 1) * 128,
                                   qtr * 1024:(qtr + 1) * 1024],
                            out_t[:, (j % 4 - 1) * 512:(j % 4 + 1) * 512])
                return emit

            def emit_attention(qt, fillers):
                qcols = slice(qt * 128, (qt + 1) * 128)
                nblk = qt + 1
                sh = shs.pop(qt)          # f32 view: attn@v accumulator
                ps_z = psZ.tile([128, HQ * 128], F32, tag="ps_z")

                def emit_S(kc):
                    ps_S = psS.tile([128, HQ * 128], F32, tag="ps_S")
                    nc.tensor.matmul(ps_S[:],
                                     kTs[:, kc * 128:(kc + 1) * 128],
                                     qTs[:, :, qcols])
                    if kc == qt:
                        nc.vector.tensor_tensor(ps_S[:], ps_S[:], maskT4[:],
                                                op=add)
                    nc.scalar.activation(
                        pT[:, kc, :, :],
                        ps_S[:].rearrange("p (h q) -> p h q", h=HQ),
                        mybir.ActivationFunctionType.Exp, bias=ebias[:])

                def emit_VZ(kc, first, last):
                    rhs = pT[:, kc, :, :].rearrange("p h q -> p (h q)")
                    nc.tensor.matmul(sh[:], v_sb[:, kc, :], rhs,
                                     start=first, stop=last)
                    nc.tensor.matmul(ps_z[:], ones_f16[:], rhs,
                                     start=first, stop=last)

                # diagonal block first: its mask-add + exp latency hides
                # behind the subsequent rounds. V/Z trail the scores by two
                # rounds so they never wait on the exp.
                ks = [qt] + list(range(nblk - 1))
                vz = 0
                emit_S(ks[0])
                for r in range(1, nblk):
                    emit_S(ks[r])
                    if r >= 2:
                        emit_VZ(ks[vz], first=(vz == 0), last=False)
                        vz += 1
                    if fillers and r % 2 == 1:
                        fillers.pop(0)()
                while vz < nblk:
                    emit_VZ(ks[vz], first=(vz == 0), last=(vz == nblk - 1))
                    vz += 1

                # normalization: zinv via 1-instruction approx reciprocal
                # (~51 ULP) on DVE, then one DVE mult
                zinv = zbuf.tile([128, HQ * 128], F32, tag="zinv")
                c = RECIP_APPROX_FAST_CONSTS
                nc.vector._custom_dve(
                    RECIPROCAL_APPROX_FAST, out=zinv[:], in0=ps_z[:],
                    s0=c["s0"], s1=c["s1"], imm2=c["imm2"])
                nc.vector.tensor_tensor(oh_all[:, qt, :], sh[:], zinv[:],
                                        op=mult)

            # ---- prologue: tile 0 through transpose ----
            emit_x_dma(0)
            emit_x_dma(1)
            nc.gpsimd.dma_start(wo_sb[:, :, 2048:], wo_r[:, :, 2048:])
            emit_proj_q(0)
            emit_rope_q(0)
            emit_proj_kv(0)
            emit_rope_k_v(0)
            emit_quant(0)
            emit_transp(0)

            # ---- fused pipeline ----
            for i in range(ST):
                fillers = [oproj_chunk(i - 1, j) for j in range(8)] \
                    if i >= 1 else []
                last = i == ST - 1
                emit_attention(i, [] if last else fillers)
                emit_x_dma(i + 2)
                if not last:
                    emit_proj_q(i + 1)
                    emit_rope_q(i + 1)
                    emit_proj_kv(i + 1)
                    emit_rope_k_v(i + 1)
                    emit_quant(i + 1)
                for f in fillers[:4]:
                    f()
                del fillers[:4]
                if not last:
                    emit_transp(i + 1)
                for f in fillers:
                    f()
                fillers.clear()
            # epilogue: last o_proj
            for j in range(8):
                oproj_chunk(ST - 1, j)()

    nc.finalize()
    return nc


_NC_CACHE = None


def _get_nc():
    global _NC_CACHE
    if _NC_CACHE is None:
        _NC_CACHE = build()
    return _NC_CACHE


def make_in_maps(x, cos, sin, Wq, Wk, Wv, Wo):
    """Shard + pre-transpose the full inputs into the 8 per-core maps.

    All layouts give the DMA large contiguous per-partition segments:
    xP[st*128+p, kc*128+s] = x[st*128+s, kc*128+p]; weights are [128, ...]
    with the SBUF-destination layout materialized host-side. The rope
    tables are packed rotate-half style: cc2 = [c; c], ss2 = [-s; s].
    """
    bf16 = ml_dtypes.bfloat16
    x = np.asarray(x, np.float32)
    xP = np.ascontiguousarray(
        x.reshape(ST, 128, KC, 128).transpose(0, 3, 2, 1)
        .reshape(ST * 128, KC * 128)).astype(bf16)
    cos = np.asarray(cos, np.float32)
    sin = np.asarray(sin, np.float32)

    def prep(t):  # [S, HD] -> [128, ST*HD]
        r = t.reshape(ST, 128, HD).transpose(1, 0, 2)
        return np.ascontiguousarray(r.reshape(128, ST * HD))

    cc2 = prep(np.concatenate([cos, cos], axis=1))
    ss2 = prep(np.concatenate([-sin, sin], axis=1))
    Wq = np.asarray(Wq, np.float32)
    Wk = np.asarray(Wk, np.float32)
    Wv = np.asarray(Wv, np.float32)
    Wo = np.asarray(Wo, np.float32)
    in_maps = []
    for c in range(N_CORES):
        qs = slice(c * HQ * HD, (c + 1) * HQ * HD)
        ks = slice(c * HD, (c + 1) * HD)
        wq_c = Wq[:, qs].reshape(KC, 128, HQ * HD).transpose(1, 0, 2)
        wkv_c = np.concatenate([Wk[:, ks], Wv[:, ks]], axis=1) \
            .reshape(KC, 128, 2 * HD).transpose(1, 0, 2)
        wo_c = Wo[qs, :].reshape(HQ, 128, D).transpose(1, 0, 2)
        in_maps.append({
            "xP": xP,
            "cc2P": cc2,
            "ss2P": ss2,
            "wqP": np.ascontiguousarray(
                wq_c.reshape(128, KC * HQ * HD)).astype(bf16),
            "wkvP": np.ascontiguousarray(
                wkv_c.reshape(128, KC * 2 * HD)).astype(bf16),
            "woP": np.ascontiguousarray(
                wo_c.reshape(128, HQ * D)).astype(np.float16),
        })
    return in_maps


def run(x, cos, sin, Wq, Wk, Wv, Wo, trace=False):
    nc = _get_nc()
    in_maps = make_in_maps(x, cos, sin, Wq, Wk, Wv, Wo)
    res = bass_utils.run_bass_kernel_spmd(
        nc, in_maps, core_ids=list(range(N_CORES)), trace=trace)
    partials = np.stack([res.results[c]["y"].astype(np.float32)
                         for c in range(N_CORES)])
    out = partials.sum(axis=0)
    return out.reshape(B, S, D), res


def kernel(x, cos, sin, Wq, Wk, Wv, Wo):
    out, _ = run(x, cos, sin, Wq, Wk, Wv, Wo, trace=False)
    return out
